# revision 18
# baseline (speedup 1.0000x reference)
"""Trainium2 Bass kernel for nn_CCA_Block (cross-channel attention block).

Reference computation (per batch element, B=8 sharded one-per-core):
    q = relu(x1 @ Wq); k = relu(x1 @ Wk); v = relu(x2 @ Wv)      # 1x1 convs
    scores[c,h,g] = scale * sum_w q[h,w,c] * k[g,w,c]
    attn = softmax(scores, axis=g)
    o[h,w,c] = sum_g attn[c,h,g] * v[g,w,c]
    g = sigmoid(o @ Ws + bs)
    g = gamma * (g - mu) / sqrt(var + eps) + beta
    out = x1 + x2 * g

Sharding: data-parallel over batch across the 8 NeuronCores (batch b -> core b).

v2 design notes (vs the first working version):
  - x1 and x2 are cast-loaded ONCE into SBUF (bf16) with 4 big chunk DMAs
    each, and stay resident: the gating multiply and the +x1 residual read
    them from SBUF instead of re-reading HBM (saves 16 MB of traffic and
    the whole SWDGE accumulate-DMA tail).
  - x1T (channel-major x1) is built by PE tile transposes during the load
    window (PE is otherwise idle there); x2T tiles are produced by DMA
    xbar transposes (sync/scalar HWDGE) while the PE runs the QK convs.
  - QK convs read strided row-gather stationaries from the column-major
    x1T, so convs need no extra data movement.
  - o_sb reuses x1T's SBUF space (same tile, dead by then).
  - Final phase is fully SBUF-resident: sigmoid -> (optional BN) ->
    t = x2*g (bf16 2x) -> out = t + x1 (fp32) split across DVE/GPSIMD,
    with plain HWDGE stores streaming out.
"""

import numpy as np
import ml_dtypes

B, H, W, C = 8, 128, 128, 128
N_CORES = 8
BN_EPS = 1e-3

_BUILD_CACHE: dict = {}


def _build_program(scale_val: float, delta: tuple, bias_via_dve: bool,
                   bn_skip: bool, b_zero: bool):
    """Emit + compile the per-core Bass program. All cores run the identical
    program on their own batch slice."""
    import concourse.bacc as bacc
    import concourse.mybir as mybir
    import concourse.tile as tile

    fp32 = mybir.dt.float32
    bf16 = mybir.dt.bfloat16
    AF = mybir.ActivationFunctionType
    OP = mybir.AluOpType
    delta_zero = all(d == 0.0 for d in delta)

    nc = bacc.Bacc("TRN2", target_bir_lowering=False, debug=False,
                   enable_asserts=False)

    x1_d = nc.dram_tensor("x1", [H, W, C], fp32, kind="ExternalInput")
    x2_d = nc.dram_tensor("x2", [H, W, C], fp32, kind="ExternalInput")
    wqk_d = nc.dram_tensor("wqk", [C, 2 * C], bf16, kind="ExternalInput")
    wv_d = nc.dram_tensor("wv", [C, C], bf16, kind="ExternalInput")
    ws_d = nc.dram_tensor("ws", [C, C], bf16, kind="ExternalInput")
    ident_d = nc.dram_tensor("ident", [C, C], bf16, kind="ExternalInput")
    arep_d = nc.dram_tensor("a_rep", [C, 4 * C], bf16, kind="ExternalInput")
    brep_d = nc.dram_tensor("b_rep", [C, 4 * C], bf16, kind="ExternalInput")
    bsrep_d = nc.dram_tensor("bs_rep", [C, 4 * C], fp32, kind="ExternalInput")
    out_d = nc.dram_tensor("out", [H, W, C], fp32, kind="ExternalOutput")

    x1_ap, x2_ap, out_ap = x1_d.ap(), x2_d.ap(), out_d.ap()

    with tile.TileContext(nc) as tc:
        with (
            # persistent single-buffer pools
            tc.tile_pool(name="wts", bufs=1) as p_wts,
            tc.tile_pool(name="res", bufs=1) as p_res,
            # streaming pools
            tc.tile_pool(name="x2T", bufs=2) as p_x2T,
            tc.tile_pool(name="eexp", bufs=5) as p_e,
            tc.tile_pool(name="rz", bufs=4) as p_rz,
            tc.tile_pool(name="oT", bufs=2) as p_oT,
            tc.tile_pool(name="gres", bufs=2) as p_g,
            tc.tile_pool(name="outt", bufs=2) as p_out,
            # psum: fp32 full banks + bf16 transpose half-banks + warm bank
            tc.tile_pool(name="psA", bufs=6, space="PSUM") as ps_a,
            tc.tile_pool(name="psT", bufs=2, space="PSUM") as ps_t,
        ):
            # ---- constants ----
            wqk = p_wts.tile([C, 2 * C], bf16, tag="wqk")
            wv = p_wts.tile([C, C], bf16, tag="wv")
            ws = p_wts.tile([C, C], bf16, tag="ws")
            ident = p_wts.tile([C, C], bf16, tag="ident")
            nc.sync.dma_start(wqk[:], wqk_d.ap())
            nc.sync.dma_start(wv[:], wv_d.ap())
            nc.sync.dma_start(ws[:], ws_d.ap())
            nc.sync.dma_start(ident[:], ident_d.ap())
            if not bn_skip:
                arep = p_wts.tile([C, 4 * C], bf16, tag="arep")
                nc.sync.dma_start(arep[:], arep_d.ap())
                if not b_zero:
                    brep = p_wts.tile([C, 4 * C], bf16, tag="brep")
                    nc.sync.dma_start(brep[:], brep_d.ap())
            if bias_via_dve:
                bsrep = p_wts.tile([C, 4 * C], fp32, tag="bsrep")
                nc.sync.dma_start(bsrep[:], bsrep_d.ap())

            # HAM keep-warm: contiguous dummy matmuls into a scratch bank
            # (never read). Transposes / strided matmuls don't register as
            # PE activity, so periodic dummies hold the clock at 2.4 GHz.
            psw = ps_a.tile([H, 512], fp32, tag="ps")

            def warm(n, wide=False):
                for _ in range(n):
                    if wide:
                        nc.tensor.matmul(psw[:], ident[:], x1_sb[:, :512],
                                         start=True, stop=True)
                    else:
                        nc.tensor.matmul(psw[:, :C], ident[:], ident[:],
                                         start=True, stop=True)

            # ---- persistent big buffers (bf16) ----
            NCH = 4
            CW = W // NCH  # 32 w per chunk
            x1_ch = [p_res.tile([H, CW * C], bf16, tag=f"x1{i}", name=f"x1c{i}") for i in range(NCH)]
            x2_ch = [p_res.tile([H, CW * C], bf16, tag=f"x2{i}", name=f"x2c{i}") for i in range(NCH)]

            def x1s(w):
                return x1_ch[w // CW][:, (w % CW) * C: (w % CW + 1) * C]

            def x2s(w):
                return x2_ch[w // CW][:, (w % CW) * C: (w % CW + 1) * C]
            # big: first x1T [c, w*128+h]; later o [h, c*128+w]
            big = p_res.tile([C, W * H], bf16, tag="big")
            q_sb = p_res.tile([W, C * H], bf16, tag="q")   # [w, c*H + h]
            k_sb = p_res.tile([W, H * C], bf16, tag="k")   # [w, h*C + c]
            q3 = q_sb[:].rearrange("w (c h) -> w c h", h=H)
            k3 = k_sb[:].rearrange("w (h c) -> w h c", c=C)
            # v stored channel-major with a trailing ones column per channel
            # [g, c*(W+1) + w]: one N=129 contiguous matmul per channel
            # yields o_unnorm plus the softmax denominator Z.
            WP = W + 1
            v_sb = p_res.tile([H, C * WP], bf16, tag="v")
            v3w = v_sb[:].rearrange("g (c wp) -> g c wp", wp=WP)
            nc.vector.memset(v3w[:, :, W], 1.0)

            # ===== loads: 4 w-chunks each, cast fp32->bf16 via SWDGE =====
            for ch in range(NCH):
                w0 = ch * CW
                nc.gpsimd.dma_start(
                    x1_ch[ch][:], x1_ap[:, w0: w0 + CW, :]
                )
                warm(20)
                # x1T tile transposes on PE (idle during loads): per w,
                # [h, c] -> [c, h]; batch 4 per bf16 PSUM half-bank.
                for p0 in range(w0, w0 + CW, 4):
                    pst = ps_t.tile([C, 512], bf16, tag="pst")
                    for j in range(4):
                        nc.tensor.matmul(
                            pst[:, j * C: (j + 1) * C], x1s(p0 + j), ident[:],
                            is_transpose=True, start=(j == 0), stop=(j == 3),
                        )
                    dst = big[:, p0 * H: (p0 + 4) * H]
                    if (p0 // 4) % 2 == 0:
                        nc.scalar.activation(dst, pst[:], AF.Copy)
                    else:
                        nc.vector.tensor_copy(dst, pst[:])
                    if p0 % 16 == 12:
                        warm(4)
            for ch in range(NCH):
                w0 = ch * CW
                nc.gpsimd.dma_start(
                    x2_ch[ch][:], x2_ap[:, w0: w0 + CW, :]
                )

            # ===== QK convs: stationary = strided row-gather from x1T =====
            # x1T layout big[c, w*H + h]; row r tile = [c, w] with stride H.
            x1T3 = big[:].rearrange("c (w h) -> c w h", h=H)
            for r0 in range(0, H, 2):
                psqk = ps_a.tile([W, 512], fp32, tag="ps")
                for t in range(2):
                    nc.tensor.matmul(
                        psqk[:, t * 256: (t + 1) * 256],
                        x1T3[:, :, r0 + t], wqk[:],
                        start=(t == 0), stop=(t == 1),
                    )
                ps4 = psqk[:].rearrange("w (t s c) -> w t s c", t=2, s=2)
                qdst = q3[:, :, r0: r0 + 2]
                qsrc = ps4.rearrange("w t s c -> w s c t")[:, 0]
                kdst = k_sb[:, r0 * C: (r0 + 2) * C]
                ksrc = ps4[:, :, 1, :]
                if (r0 // 2) % 2 == 0:
                    nc.scalar.activation(qdst, qsrc, AF.Relu)
                    nc.vector.tensor_scalar(kdst, ksrc, 0.0, None, OP.max)
                else:
                    nc.vector.tensor_scalar(qdst, qsrc, 0.0, None, OP.max)
                    nc.scalar.activation(kdst, ksrc, AF.Relu)

            # ===== V convs: x2T tiles via PE transpose =====
            for p0 in range(0, W, 4):
                pst = ps_t.tile([C, 512], bf16, tag="pst")
                for j in range(4):
                    nc.tensor.matmul(
                        pst[:, j * C: (j + 1) * C], x2s(p0 + j), ident[:],
                        is_transpose=True, start=(j == 0), stop=(j == 3),
                    )
                x2T = p_x2T.tile([C, 512], bf16, tag="x2T")
                if (p0 // 4) % 2 == 0:
                    nc.scalar.activation(x2T[:], pst[:], AF.Copy)
                else:
                    nc.vector.tensor_copy(x2T[:], pst[:])
                psv = ps_a.tile([H, 512], fp32, tag="ps")
                for j in range(4):
                    nc.tensor.matmul(
                        psv[:, j * C: (j + 1) * C],
                        x2T[:, j * C: (j + 1) * C], wv[:],
                        start=(j == 0), stop=(j == 3),
                    )
                # src iterates (c, j): strided source, contiguous dest runs
                dst = v3w[:, :, p0: p0 + 4]  # ones col at index W untouched
                src = psv[:].rearrange("g (j c) -> g c j", c=C)
                if (p0 // 4) % 2 == 0:
                    nc.vector.tensor_scalar(dst, src, 0.0, None, OP.max)
                else:
                    nc.scalar.activation(dst, src, AF.Relu)

            # ============ attention over channels (3-channel groups) =======
            groups = [(c0, min(2, C - c0)) for c0 in range(0, C, 2)]
            for c0, gs in groups:
                pss = ps_a.tile([H, gs * H], fp32, tag="ps")
                for j in range(gs):
                    c = c0 + j
                    nc.tensor.matmul(
                        pss[:, j * H: (j + 1) * H],
                        k3[:, :, c], q_sb[:, c * H: (c + 1) * H],
                        start=(j == 0), stop=(j == gs - 1),
                    )
                e4 = p_e.tile([H, gs * H], bf16, tag="e4")
                nc.scalar.activation(e4[:], pss[:], AF.Exp, scale=scale_val)
                pso = ps_a.tile([H, gs * 129], fp32, tag="ps")
                for j in range(gs):
                    c = c0 + j
                    nc.tensor.matmul(
                        pso[:, j * 129: (j + 1) * 129],
                        e4[:, j * H: (j + 1) * H],
                        v_sb[:, c * WP: (c + 1) * WP],
                        start=(j == 0), stop=(j == gs - 1),
                    )
                po = pso[:].rearrange("h (j x) -> h j x", x=129)
                rz = p_rz.tile([H, gs], fp32, tag="rz")
                nc.vector.reciprocal(rz[:], po[:, :, 128])
                # o written into big (x1T space): layout [h, w*C + c]
                ow3 = big[:].rearrange("h (w c) -> h w c", c=C)
                if delta_zero:
                    dst = ow3[:, :, c0: c0 + gs]
                    src = po[:, :, :W].rearrange("h j x -> h x j")
                    rzb = rz[:].unsqueeze(1).broadcast_to([H, C, gs])
                    nc.vector.tensor_tensor(dst, src, rzb, OP.mult)
                else:
                    for j in range(gs):
                        c = c0 + j
                        dst = ow3[:, :, c]
                        src_ap = pso[:, j * 129: j * 129 + W]
                        nc.vector.tensor_scalar(
                            dst, src_ap, rz[:, j: j + 1], float(delta[c]),
                            OP.mult, OP.add,
                        )

            # ============ G: oT -> conv -> sigmoid/BN -> gated residual ====
            for w0 in range(0, W, 4):
                pst = ps_t.tile([C, 512], bf16, tag="pst")
                for j in range(4):
                    nc.tensor.matmul(
                        pst[:, j * C: (j + 1) * C],
                        big[:, (w0 + j) * C: (w0 + j + 1) * C], ident[:],
                        is_transpose=True, start=(j == 0), stop=(j == 3),
                    )
                oT = (p_oT if (w0 // 4) % 2 == 0 else p_x2T).tile(
                    [C, 512], bf16, tag="oT2" if (w0 // 4) % 2 == 0 else "x2T"
                )
                nc.scalar.activation(oT[:, :256], pst[:, :256], AF.Copy)
                nc.vector.tensor_copy(oT[:, 256:], pst[:, 256:])
                psg = ps_a.tile([H, 512], fp32, tag="ps")
                for j in range(4):
                    nc.tensor.matmul(
                        psg[:, j * C: (j + 1) * C],
                        oT[:, j * H: (j + 1) * H], ws[:],
                        start=(j == 0), stop=(j == 3),
                    )
                if bias_via_dve:
                    nc.vector.tensor_tensor(psg[:], psg[:], bsrep[:], OP.add)
                g4 = p_g.tile([H, 512], bf16, tag="g4")
                nc.scalar.activation(g4[:], psg[:], AF.Sigmoid)
                if not bn_skip:
                    nc.vector.tensor_tensor(g4[:], g4[:], arep[:], OP.mult)
                    if not b_zero:
                        nc.vector.tensor_tensor(g4[:], g4[:], brep[:], OP.add)
                # t = x2 * g, in place into g4 (bf16, 2x mode)
                x2sl = x2_ch[w0 // CW][:, (w0 % CW) * C: (w0 % CW + 4) * C]
                x1sl = x1_ch[w0 // CW][:, (w0 % CW) * C: (w0 % CW + 4) * C]
                gi = w0 // 4
                if gi % 3 == 1:
                    nc.gpsimd.tensor_tensor(g4[:], x2sl, g4[:], OP.mult)
                else:
                    nc.vector.tensor_tensor(g4[:], x2sl, g4[:], OP.mult)
                # out = t + x1 (fp32), split DVE / gpsimd
                outt = p_out.tile([H, 512], fp32, tag="outt")
                if gi % 3 == 0:
                    nc.gpsimd.tensor_tensor(outt[:], x1sl, g4[:], OP.add)
                else:
                    nc.vector.tensor_tensor(outt[:], x1sl, g4[:], OP.add)
                nc.sync.dma_start(out_ap[:, w0: w0 + 4, :], outt[:])

    nc.compile()
    return nc


def _prepare(inputs):
    """Host-side prep: derived small tensors + baked scalars."""
    x1 = np.ascontiguousarray(np.asarray(inputs["x1"], dtype=np.float32))
    x2 = np.ascontiguousarray(np.asarray(inputs["x2"], dtype=np.float32))
    Wq = np.asarray(inputs["Wq"], dtype=np.float32)
    Wk = np.asarray(inputs["Wk"], dtype=np.float32)
    Wv = np.asarray(inputs["Wv"], dtype=np.float32)
    Ws = np.asarray(inputs["Ws"], dtype=np.float32)
    bs = np.asarray(inputs["bs"], dtype=np.float32)
    scale = float(np.asarray(inputs["scale"]).reshape(-1)[0])
    gamma = np.asarray(inputs["gamma"], dtype=np.float32)
    beta = np.asarray(inputs["beta"], dtype=np.float32)
    mu = np.asarray(inputs["mu"], dtype=np.float32)
    var = np.asarray(inputs["var"], dtype=np.float32)

    a = gamma / np.sqrt(var + BN_EPS)
    b = beta - mu * a
    b_zero = bool(np.all(b == 0.0))
    # BN is a near-identity in practice; skipping it keeps the whole gating
    # path on two DVE ops. Error bound: |x2|max * (|a-1| + |b|) << tol.
    bn_skip = bool(np.abs(a - 1.0).max() < 1.5e-3 and np.abs(b).max() < 1.5e-3)

    # fold the sigmoid bias bs into o:  o' = o + delta with Ws^T delta = bs
    bias_via_dve = False
    delta = np.zeros(C, dtype=np.float64)
    if np.any(bs != 0.0):
        try:
            delta = np.linalg.solve(Ws.astype(np.float64).T, bs.astype(np.float64))
            resid = np.abs(Ws.T @ delta.astype(np.float32) - bs).max()
            if not np.isfinite(delta).all() or resid > 1e-5 * (1 + np.abs(bs).max()):
                raise np.linalg.LinAlgError("bad solve")
        except np.linalg.LinAlgError:
            delta = np.zeros(C, dtype=np.float64)
            bias_via_dve = True

    bf = ml_dtypes.bfloat16
    consts = {
        "wqk": np.concatenate([Wq, Wk], axis=1).astype(bf),
        "wv": Wv.astype(bf),
        "ws": Ws.astype(bf),
        "ident": np.eye(C, dtype=bf),
        "a_rep": np.tile(a, (C, 4)).astype(bf),
        "b_rep": np.tile(b, (C, 4)).astype(bf),
        "bs_rep": np.tile(bs, (C, 4)).astype(np.float32),
    }
    key = (scale, tuple(np.round(delta, 12)), bias_via_dve, bn_skip, b_zero)
    return x1, x2, consts, key, scale, delta, bias_via_dve, bn_skip, b_zero


def _get_nc(key, scale, delta, bias_via_dve, bn_skip, b_zero):
    if key not in _BUILD_CACHE:
        _BUILD_CACHE[key] = _build_program(
            scale, delta, bias_via_dve, bn_skip, b_zero
        )
    return _BUILD_CACHE[key]


def run(inputs, trace: bool = False):
    from concourse.bass_utils import run_bass_kernel_spmd

    x1, x2, consts, key, scale, delta, bias_via_dve, bn_skip, b_zero = _prepare(
        inputs
    )
    nc = _get_nc(key, scale, delta, bias_via_dve, bn_skip, b_zero)

    in_maps = []
    for core in range(N_CORES):
        m = dict(consts)
        m["x1"] = x1[core]
        m["x2"] = x2[core]
        in_maps.append(m)

    res = run_bass_kernel_spmd(
        nc, in_maps, core_ids=list(range(N_CORES)), trace=trace
    )
    out = np.stack([res.results[i]["out"] for i in range(N_CORES)], axis=0)
    return out.astype(np.float32), res


def kernel(**inputs) -> np.ndarray:
    out, _ = run(inputs, trace=False)
    return out


# revision 20
# speedup vs baseline: 1.2016x; 1.2016x over previous
"""Trainium2 Bass kernel for nn_CCA_Block (cross-channel attention block).

Reference computation (per batch element, B=8 sharded one-per-core):
    q = relu(x1 @ Wq); k = relu(x1 @ Wk); v = relu(x2 @ Wv)      # 1x1 convs
    scores[c,h,g] = scale * sum_w q[h,w,c] * k[g,w,c]
    attn = softmax(scores, axis=g)
    o[h,w,c] = sum_g attn[c,h,g] * v[g,w,c]
    g = sigmoid(o @ Ws + bs)
    g = gamma * (g - mu) / sqrt(var + eps) + beta
    out = x1 + x2 * g

Sharding: data-parallel over batch across the 8 NeuronCores (batch b -> core b).

v2 design notes (vs the first working version):
  - x1 and x2 are cast-loaded ONCE into SBUF (bf16) with 4 big chunk DMAs
    each, and stay resident: the gating multiply and the +x1 residual read
    them from SBUF instead of re-reading HBM (saves 16 MB of traffic and
    the whole SWDGE accumulate-DMA tail).
  - x1T (channel-major x1) is built by PE tile transposes during the load
    window (PE is otherwise idle there); x2T tiles are produced by DMA
    xbar transposes (sync/scalar HWDGE) while the PE runs the QK convs.
  - QK convs read strided row-gather stationaries from the column-major
    x1T, so convs need no extra data movement.
  - o_sb reuses x1T's SBUF space (same tile, dead by then).
  - Final phase is fully SBUF-resident: sigmoid -> (optional BN) ->
    t = x2*g (bf16 2x) -> out = t + x1 (fp32) split across DVE/GPSIMD,
    with plain HWDGE stores streaming out.
"""

import numpy as np
import ml_dtypes

B, H, W, C = 8, 128, 128, 128
N_CORES = 8
BN_EPS = 1e-3

_BUILD_CACHE: dict = {}


def _build_program(scale_val: float, delta: tuple, bias_via_dve: bool,
                   bn_skip: bool, b_zero: bool):
    """Emit + compile the per-core Bass program. All cores run the identical
    program on their own batch slice."""
    import concourse.bacc as bacc
    import concourse.mybir as mybir
    import concourse.tile as tile

    fp32 = mybir.dt.float32
    bf16 = mybir.dt.bfloat16
    AF = mybir.ActivationFunctionType
    OP = mybir.AluOpType
    delta_zero = all(d == 0.0 for d in delta)

    nc = bacc.Bacc("TRN2", target_bir_lowering=False, debug=False,
                   enable_asserts=False)

    x1_d = nc.dram_tensor("x1", [H, W, C], fp32, kind="ExternalInput")
    x2_d = nc.dram_tensor("x2", [H, W, C], fp32, kind="ExternalInput")
    wqk_d = nc.dram_tensor("wqk", [C, 2 * C], bf16, kind="ExternalInput")
    wv_d = nc.dram_tensor("wv", [C, C], bf16, kind="ExternalInput")
    ws_d = nc.dram_tensor("ws", [C, C], bf16, kind="ExternalInput")
    ident_d = nc.dram_tensor("ident", [C, C], bf16, kind="ExternalInput")
    arep_d = nc.dram_tensor("a_rep", [C, 4 * C], bf16, kind="ExternalInput")
    brep_d = nc.dram_tensor("b_rep", [C, 4 * C], bf16, kind="ExternalInput")
    bsrep_d = nc.dram_tensor("bs_rep", [C, 4 * C], fp32, kind="ExternalInput")
    out_d = nc.dram_tensor("out", [H, W, C], fp32, kind="ExternalOutput")

    x1_ap, x2_ap, out_ap = x1_d.ap(), x2_d.ap(), out_d.ap()

    with tile.TileContext(nc) as tc:
        with (
            # persistent single-buffer pools
            tc.tile_pool(name="wts", bufs=1) as p_wts,
            tc.tile_pool(name="res", bufs=1) as p_res,
            # streaming pools
            tc.tile_pool(name="x2T", bufs=2) as p_x2T,
            tc.tile_pool(name="eexp", bufs=5) as p_e,
            tc.tile_pool(name="rz", bufs=4) as p_rz,
            tc.tile_pool(name="oT", bufs=2) as p_oT,
            tc.tile_pool(name="gres", bufs=2) as p_g,
            tc.tile_pool(name="outt", bufs=2) as p_out,
            # psum: fp32 full banks + bf16 transpose half-banks + warm bank
            tc.tile_pool(name="psA", bufs=6, space="PSUM") as ps_a,
            tc.tile_pool(name="psT", bufs=2, space="PSUM") as ps_t,
        ):
            # ---- constants ----
            wqk = p_wts.tile([C, 2 * C], bf16, tag="wqk")
            wv = p_wts.tile([C, C], bf16, tag="wv")
            ws = p_wts.tile([C, C], bf16, tag="ws")
            ident = p_wts.tile([C, C], bf16, tag="ident")
            nc.sync.dma_start(wqk[:], wqk_d.ap())
            nc.sync.dma_start(wv[:], wv_d.ap())
            nc.sync.dma_start(ws[:], ws_d.ap())
            nc.sync.dma_start(ident[:], ident_d.ap())
            if not bn_skip:
                arep = p_wts.tile([C, 4 * C], bf16, tag="arep")
                nc.sync.dma_start(arep[:], arep_d.ap())
                if not b_zero:
                    brep = p_wts.tile([C, 4 * C], bf16, tag="brep")
                    nc.sync.dma_start(brep[:], brep_d.ap())
            if bias_via_dve:
                bsrep = p_wts.tile([C, 4 * C], fp32, tag="bsrep")
                nc.sync.dma_start(bsrep[:], bsrep_d.ap())

            # HAM keep-warm: contiguous dummy matmuls into a scratch bank
            # (never read). Transposes / strided matmuls don't register as
            # PE activity, so periodic dummies hold the clock at 2.4 GHz.
            psw = ps_a.tile([H, 512], fp32, tag="ps")

            def warm(n, wide=False):
                for _ in range(n):
                    if wide:
                        nc.tensor.matmul(psw[:], ident[:], x1_sb[:, :512],
                                         start=True, stop=True)
                    else:
                        nc.tensor.matmul(psw[:, :C], ident[:], ident[:],
                                         start=True, stop=True)

            # ---- persistent big buffers (bf16) ----
            NCH = 4
            CW = W // NCH  # 32 w per chunk
            x1_ch = [p_res.tile([H, CW * C], bf16, tag=f"x1{i}", name=f"x1c{i}") for i in range(NCH)]
            x2_ch = [p_res.tile([H, CW * C], bf16, tag=f"x2{i}", name=f"x2c{i}") for i in range(NCH)]

            def x1s(w):
                return x1_ch[w // CW][:, (w % CW) * C: (w % CW + 1) * C]

            def x2s(w):
                return x2_ch[w // CW][:, (w % CW) * C: (w % CW + 1) * C]
            # big: first x1T [c, w*128+h]; later o [h, c*128+w]
            big = p_res.tile([C, W * H], bf16, tag="big")
            q_sb = p_res.tile([W, C * H], bf16, tag="q")   # [w, c*H + h]
            k_sb = p_res.tile([W, H * C], bf16, tag="k")   # [w, h*C + c]
            q3 = q_sb[:].rearrange("w (c h) -> w c h", h=H)
            k3 = k_sb[:].rearrange("w (h c) -> w h c", c=C)
            # v stored channel-major with a trailing ones column per channel
            # [g, c*(W+1) + w]: one N=129 contiguous matmul per channel
            # yields o_unnorm plus the softmax denominator Z.
            WP = W + 1
            v_sb = p_res.tile([H, C * WP], bf16, tag="v")
            v3w = v_sb[:].rearrange("g (c wp) -> g c wp", wp=WP)
            nc.vector.memset(v3w[:, :, W], 1.0)

            # ===== loads: 4 w-chunks each, cast fp32->bf16 via SWDGE =====
            for ch in range(NCH):
                w0 = ch * CW
                nc.gpsimd.dma_start(
                    x1_ch[ch][:], x1_ap[:, w0: w0 + CW, :]
                )
                warm(20)
                # x1T tile transposes on PE (idle during loads): per w,
                # [h, c] -> [c, h]; batch 4 per bf16 PSUM half-bank.
                for p0 in range(w0, w0 + CW, 4):
                    pst = ps_t.tile([C, 512], bf16, tag="pst")
                    for j in range(4):
                        nc.tensor.matmul(
                            pst[:, j * C: (j + 1) * C], x1s(p0 + j), ident[:],
                            is_transpose=True, start=(j == 0), stop=(j == 3),
                        )
                    dst = big[:, p0 * H: (p0 + 4) * H]
                    if (p0 // 4) % 2 == 0:
                        nc.scalar.activation(dst, pst[:], AF.Copy)
                    else:
                        nc.vector.tensor_copy(dst, pst[:])
                    if p0 % 16 == 12:
                        warm(4)
            for ch in range(NCH):
                w0 = ch * CW
                nc.gpsimd.dma_start(
                    x2_ch[ch][:], x2_ap[:, w0: w0 + CW, :]
                )

            # ===== QK convs: stationary = strided row-gather from x1T =====
            # x1T layout big[c, w*H + h]; row r tile = [c, w] with stride H.
            x1T3 = big[:].rearrange("c (w h) -> c w h", h=H)
            for r0 in range(0, H, 2):
                psqk = ps_a.tile([W, 512], fp32, tag="ps")
                for t in range(2):
                    nc.tensor.matmul(
                        psqk[:, t * 256: (t + 1) * 256],
                        x1T3[:, :, r0 + t], wqk[:],
                        start=(t == 0), stop=(t == 1),
                    )
                ps4 = psqk[:].rearrange("w (t s c) -> w t s c", t=2, s=2)
                qdst = q3[:, :, r0: r0 + 2]
                qsrc = ps4.rearrange("w t s c -> w s c t")[:, 0]
                kdst = k_sb[:, r0 * C: (r0 + 2) * C]
                ksrc = ps4[:, :, 1, :]
                if (r0 // 2) % 2 == 0:
                    nc.scalar.activation(qdst, qsrc, AF.Relu)
                    nc.vector.tensor_scalar(kdst, ksrc, 0.0, None, OP.max)
                else:
                    nc.vector.tensor_scalar(qdst, qsrc, 0.0, None, OP.max)
                    nc.scalar.activation(kdst, ksrc, AF.Relu)

            # ===== V convs: x2T tiles via PE transpose =====
            for p0 in range(0, W, 4):
                pst = ps_t.tile([C, 512], bf16, tag="pst")
                for j in range(4):
                    nc.tensor.matmul(
                        pst[:, j * C: (j + 1) * C], x2s(p0 + j), ident[:],
                        is_transpose=True, start=(j == 0), stop=(j == 3),
                    )
                x2T = p_x2T.tile([C, 512], bf16, tag="x2T")
                if (p0 // 4) % 2 == 0:
                    nc.scalar.activation(x2T[:], pst[:], AF.Copy)
                else:
                    nc.vector.tensor_copy(x2T[:], pst[:])
                psv = ps_a.tile([H, 512], fp32, tag="ps")
                for j in range(4):
                    nc.tensor.matmul(
                        psv[:, j * C: (j + 1) * C],
                        x2T[:, j * C: (j + 1) * C], wv[:],
                        start=(j == 0), stop=(j == 3),
                    )
                # src iterates (c, j): strided source, contiguous dest runs
                dst = v3w[:, :, p0: p0 + 4]  # ones col at index W untouched
                src = psv[:].rearrange("g (j c) -> g c j", c=C)
                if (p0 // 4) % 2 == 0:
                    nc.vector.tensor_scalar(dst, src, 0.0, None, OP.max)
                else:
                    nc.scalar.activation(dst, src, AF.Relu)

            # ============ attention over channels (3-channel groups) =======
            groups = [(c0, min(3, C - c0)) for c0 in range(0, C, 3)]
            for c0, gs in groups:
                psb = ps_a.tile([H, 512], fp32, tag="ps")
                pss = psb[:, : gs * H]
                for j in range(gs):
                    c = c0 + j
                    nc.tensor.matmul(
                        psb[:, j * H: (j + 1) * H],
                        k3[:, :, c], q_sb[:, c * H: (c + 1) * H],
                        start=(j == 0), stop=(j == gs - 1),
                    )
                e4 = p_e.tile([H, gs * H], bf16, tag="e4")
                nc.scalar.activation(e4[:], pss, AF.Exp, scale=scale_val)
                pso = psb
                for j in range(gs):
                    c = c0 + j
                    nc.tensor.matmul(
                        pso[:, j * 129: (j + 1) * 129],
                        e4[:, j * H: (j + 1) * H],
                        v_sb[:, c * WP: (c + 1) * WP],
                        start=(j == 0), stop=(j == gs - 1),
                    )
                po = pso[:, : gs * 129].rearrange("h (j x) -> h j x", x=129)
                rz = p_rz.tile([H, gs], fp32, tag="rz")
                nc.vector.reciprocal(rz[:], po[:, :, 128])
                # o written into big (x1T space): layout [h, w*C + c]
                ow3 = big[:].rearrange("h (w c) -> h w c", c=C)
                if delta_zero:
                    dst = ow3[:, :, c0: c0 + gs]
                    src = po[:, :, :W].rearrange("h j x -> h x j")
                    rzb = rz[:].unsqueeze(1).broadcast_to([H, C, gs])
                    nc.vector.tensor_tensor(dst, src, rzb, OP.mult)
                else:
                    for j in range(gs):
                        c = c0 + j
                        dst = ow3[:, :, c]
                        src_ap = pso[:, j * 129: j * 129 + W]
                        nc.vector.tensor_scalar(
                            dst, src_ap, rz[:, j: j + 1], float(delta[c]),
                            OP.mult, OP.add,
                        )

            # ============ G: oT -> conv -> sigmoid/BN -> gated residual ====
            for w0 in range(0, W, 4):
                pst = ps_t.tile([C, 512], bf16, tag="pst")
                for j in range(4):
                    nc.tensor.matmul(
                        pst[:, j * C: (j + 1) * C],
                        big[:, (w0 + j) * C: (w0 + j + 1) * C], ident[:],
                        is_transpose=True, start=(j == 0), stop=(j == 3),
                    )
                oT = (p_oT if (w0 // 4) % 2 == 0 else p_x2T).tile(
                    [C, 512], bf16, tag="oT2" if (w0 // 4) % 2 == 0 else "x2T"
                )
                nc.scalar.activation(oT[:, :256], pst[:, :256], AF.Copy)
                nc.vector.tensor_copy(oT[:, 256:], pst[:, 256:])
                psg = ps_a.tile([H, 512], fp32, tag="ps")
                for j in range(4):
                    nc.tensor.matmul(
                        psg[:, j * C: (j + 1) * C],
                        oT[:, j * H: (j + 1) * H], ws[:],
                        start=(j == 0), stop=(j == 3),
                    )
                if bias_via_dve:
                    nc.vector.tensor_tensor(psg[:], psg[:], bsrep[:], OP.add)
                g4 = p_g.tile([H, 512], bf16, tag="g4")
                nc.scalar.activation(g4[:], psg[:], AF.Sigmoid)
                if not bn_skip:
                    nc.vector.tensor_tensor(g4[:], g4[:], arep[:], OP.mult)
                    if not b_zero:
                        nc.vector.tensor_tensor(g4[:], g4[:], brep[:], OP.add)
                # t = x2 * g, in place into g4 (bf16, 2x mode)
                x2sl = x2_ch[w0 // CW][:, (w0 % CW) * C: (w0 % CW + 4) * C]
                x1sl = x1_ch[w0 // CW][:, (w0 % CW) * C: (w0 % CW + 4) * C]
                gi = w0 // 4
                if gi % 3 == 1:
                    nc.gpsimd.tensor_tensor(g4[:], x2sl, g4[:], OP.mult)
                else:
                    nc.vector.tensor_tensor(g4[:], x2sl, g4[:], OP.mult)
                # out = t + x1 (fp32), split DVE / gpsimd
                outt = p_out.tile([H, 512], fp32, tag="outt")
                if gi % 3 == 0:
                    nc.gpsimd.tensor_tensor(outt[:], x1sl, g4[:], OP.add)
                else:
                    nc.vector.tensor_tensor(outt[:], x1sl, g4[:], OP.add)
                nc.sync.dma_start(out_ap[:, w0: w0 + 4, :], outt[:])

    nc.compile()
    return nc


def _prepare(inputs):
    """Host-side prep: derived small tensors + baked scalars."""
    x1 = np.ascontiguousarray(np.asarray(inputs["x1"], dtype=np.float32))
    x2 = np.ascontiguousarray(np.asarray(inputs["x2"], dtype=np.float32))
    Wq = np.asarray(inputs["Wq"], dtype=np.float32)
    Wk = np.asarray(inputs["Wk"], dtype=np.float32)
    Wv = np.asarray(inputs["Wv"], dtype=np.float32)
    Ws = np.asarray(inputs["Ws"], dtype=np.float32)
    bs = np.asarray(inputs["bs"], dtype=np.float32)
    scale = float(np.asarray(inputs["scale"]).reshape(-1)[0])
    gamma = np.asarray(inputs["gamma"], dtype=np.float32)
    beta = np.asarray(inputs["beta"], dtype=np.float32)
    mu = np.asarray(inputs["mu"], dtype=np.float32)
    var = np.asarray(inputs["var"], dtype=np.float32)

    a = gamma / np.sqrt(var + BN_EPS)
    b = beta - mu * a
    b_zero = bool(np.all(b == 0.0))
    # BN is a near-identity in practice; skipping it keeps the whole gating
    # path on two DVE ops. Error bound: |x2|max * (|a-1| + |b|) << tol.
    bn_skip = bool(np.abs(a - 1.0).max() < 1.5e-3 and np.abs(b).max() < 1.5e-3)

    # fold the sigmoid bias bs into o:  o' = o + delta with Ws^T delta = bs
    bias_via_dve = False
    delta = np.zeros(C, dtype=np.float64)
    if np.any(bs != 0.0):
        try:
            delta = np.linalg.solve(Ws.astype(np.float64).T, bs.astype(np.float64))
            resid = np.abs(Ws.T @ delta.astype(np.float32) - bs).max()
            if not np.isfinite(delta).all() or resid > 1e-5 * (1 + np.abs(bs).max()):
                raise np.linalg.LinAlgError("bad solve")
        except np.linalg.LinAlgError:
            delta = np.zeros(C, dtype=np.float64)
            bias_via_dve = True

    bf = ml_dtypes.bfloat16
    consts = {
        "wqk": np.concatenate([Wq, Wk], axis=1).astype(bf),
        "wv": Wv.astype(bf),
        "ws": Ws.astype(bf),
        "ident": np.eye(C, dtype=bf),
        "a_rep": np.tile(a, (C, 4)).astype(bf),
        "b_rep": np.tile(b, (C, 4)).astype(bf),
        "bs_rep": np.tile(bs, (C, 4)).astype(np.float32),
    }
    key = (scale, tuple(np.round(delta, 12)), bias_via_dve, bn_skip, b_zero)
    return x1, x2, consts, key, scale, delta, bias_via_dve, bn_skip, b_zero


def _get_nc(key, scale, delta, bias_via_dve, bn_skip, b_zero):
    if key not in _BUILD_CACHE:
        _BUILD_CACHE[key] = _build_program(
            scale, delta, bias_via_dve, bn_skip, b_zero
        )
    return _BUILD_CACHE[key]


def run(inputs, trace: bool = False):
    from concourse.bass_utils import run_bass_kernel_spmd

    x1, x2, consts, key, scale, delta, bias_via_dve, bn_skip, b_zero = _prepare(
        inputs
    )
    nc = _get_nc(key, scale, delta, bias_via_dve, bn_skip, b_zero)

    in_maps = []
    for core in range(N_CORES):
        m = dict(consts)
        m["x1"] = x1[core]
        m["x2"] = x2[core]
        in_maps.append(m)

    res = run_bass_kernel_spmd(
        nc, in_maps, core_ids=list(range(N_CORES)), trace=trace
    )
    out = np.stack([res.results[i]["out"] for i in range(N_CORES)], axis=0)
    return out.astype(np.float32), res


def kernel(**inputs) -> np.ndarray:
    out, _ = run(inputs, trace=False)
    return out
